# revision 1
# baseline (speedup 1.0000x reference)
"""Trainium2 Bass kernel for ContourIntegrationLayer.

Math: out = x + depthwise_corr5x5(x, k) on NHWC x:(128,55,55,96), k:(96,5,5).
Only 4 channels of k are nonzero: 5, 10 (cross pattern, opposite signs) and
54, 67 (identical diagonal pattern).

Strategy (pure data parallel over batch, 16 images/core):
  - Per core, stream 8-image tiles [110 part = (img-parity, h), 4*55*96
    free = (img-pair j, w, c)] through SBUF with large contiguous DMAs
    (only 5 DMAs per core total -> no DMA-lane or SBUF-slot reuse, which
    keeps every instruction under walrus's sync-wait limit).
  - The 5x5 stencil is grouped by dw (horizontal tap offset). For each dw
    the vertical structure is a small 55x55 banded matrix S applied on the
    partition (h) axis -> TensorE matmul with S (block-diag over 2 images)
    as stationary weights and a strided view of the tile (channel pair
    columns, dw-shifted in w, all 4 image-pairs at once) as the moving
    operand, accumulating the taps in PSUM. Horizontal SAME-padding is
    handled by shrinking the w range of the dw!=0 terms (their
    out-of-range contribution is zero); the dw=0 term covers the full
    range first (start=True), so every PSUM element is initialized.
  - 4 strided DVE tensor_add/sub ops merge the per-channel deltas into the
    tile in place; the whole tile is then DMA'd back out.
Memory traffic is the roofline: read x once + write out once per core.
"""

import numpy as np

try:
    import concourse.bass as bass  # noqa: F401
except ImportError:  # harness runs in a fresh dir; repo is at a fixed path
    import sys

    sys.path.insert(0, "/opt/trn_rl_repo")

import concourse.bacc as bacc
import concourse.bass as bass  # noqa: F401
import concourse.mybir as mybir
import concourse.tile as tile
from concourse.bass_utils import run_bass_kernel_spmd

N_CORES = 8
H = W = 55
C = 96
FREE1 = W * C         # 5280 elements per image row-block
ROWS = 2 * H          # 110 partitions: two images interleaved on partitions
CROSS_CH = (5, 10)    # k[5] = -P, k[10] = +P
DIAG_CH = (54, 67)    # k[54] = k[67] = Q
DWS = (0, -2, -1, 1, 2)   # dw=0 first: full w coverage, starts the group
N_MATS = 10           # 5 cross + 5 diag dw-terms (diag dw=0 is all zeros)


def build_nc(n_images: int, ipt: int = 2, repeats: int = 1, mode: str = 'full'):
    """Per-core Bass program; ipt = images per SBUF tile (even, <= 8).

    repeats > 1 re-runs the whole pass (same input -> same output) for
    dispatch-overhead-free timing via (T(R2)-T(R1))/(R2-R1).
    """
    ipt = min(ipt, n_images)
    assert n_images % ipt == 0 and ipt % 2 == 0
    n_tiles = n_images // ipt
    jj = ipt // 2           # image pairs per tile (free-dim chunks)
    # Bacc (not raw Bass): its finalize() runs generate_event_semaphores,
    # which splits multi-sem waits down to the 1-wait-per-instruction TRN2
    # limit that walrus enforces.
    nc = bacc.Bacc()
    x_in = nc.dram_tensor("x", [n_images, H, W, C], mybir.dt.float32,
                          kind="ExternalInput")
    s_in = nc.dram_tensor("s_mats", [ROWS, N_MATS * ROWS], mybir.dt.float32,
                          kind="ExternalInput")
    out = nc.dram_tensor("out", [n_images, H, W, C], mybir.dt.float32,
                         kind="ExternalOutput")

    # row g of the flat view = (image n, h); tile t, partition p=(i,h), free
    # chunk j <-> image 2*jj*t + 2*j + i, rows of one image pair contiguous
    xd = x_in[:].rearrange("(t j p) h w c -> t (p h) j (w c)", t=n_tiles, j=jj, p=2)
    od = out[:].rearrange("(t j p) h w c -> t (p h) j (w c)", t=n_tiles, j=jj, p=2)

    with tile.TileContext(nc) as tc:
        with (
            tc.tile_pool(name="const", bufs=1) as cpool,
            tc.tile_pool(name="work", bufs=min(4, n_tiles)) as pool,
            tc.tile_pool(name="psum", bufs=min(2, n_tiles), space="PSUM") as psum,
            tc.tile_pool(name="psumd", bufs=1, space="PSUM") as psumd,
        ):
            s_sb = cpool.tile([ROWS, N_MATS * ROWS], mybir.dt.float32)
            nc.sync.dma_start(out=s_sb[:], in_=s_in[:])

            # dummy matmul reading only s_mats: absorbs the s_mats DMA wait
            # so the first real matmul needs just one wait (walrus allows a
            # single sync wait per Matmult)
            pd = psumd.tile([ROWS, 1], mybir.dt.float32, name="pd", tag="pd")
            nc.tensor.matmul(pd[:], s_sb[:, 0:ROWS], s_sb[:, 0:1],
                             start=True, stop=True)

            import contextlib

            loop = tc.For_i(0, repeats, 1) if repeats > 1 else contextlib.nullcontext()
            with loop:
                _body(nc, tc, pool, psum, s_sb, xd, od, n_tiles, jj, mode)
    nc.finalize()
    return nc


def _body(nc, tc, pool, psum, s_sb, xd, od, n_tiles, jj, mode='full'):
    for t in range(n_tiles):
        xt = pool.tile([ROWS, jj * FREE1], mybir.dt.float32, tag="xt")
        nc.sync.dma_start(out=xt[:], in_=xd[t])
        xv = xt[:].rearrange("p (j w c) -> p j w c", j=jj, c=C)

        if mode == 'dma':
            nc.sync.dma_start(out=od[t], in_=xt[:])
            continue
        pa = psum.tile([ROWS, jj * 2 * W], mybir.dt.float32,
                       name=f"pa{t}", tag="pa")
        pb = psum.tile([ROWS, jj * 2 * W], mybir.dt.float32,
                       name=f"pb{t}", tag="pb")
        pav = pa[:].rearrange("p (j w c) -> p j w c", j=jj, c=2)
        pbv = pb[:].rearrange("p (j w c) -> p j w c", j=jj, c=2)
        for grp, (pv, (c0, c1)) in enumerate(
            ((pav, CROSS_CH), (pbv, DIAG_CH))
        ):
            st = c1 - c0
            for j, dw in enumerate(DWS):
                cnt = W - abs(dw)
                wo = max(0, -dw)          # first valid out w
                wi = wo + dw              # first read w
                rhs = xv[:, :, wi : wi + cnt, c0 : c1 + 1 : st]
                nc.tensor.matmul(
                    pv[:, :, wo : wo + cnt, :],
                    s_sb[:, ROWS * (5 * grp + j) : ROWS * (5 * grp + j + 1)],
                    rhs,
                    start=(j == 0),
                    stop=(j == len(DWS) - 1),
                )
        # 1-element DVE read of xt: absorbs the load-DMA wait so each
        # merge below needs at most one sync wait (walrus limit)
        gk = pool.tile([ROWS, 1], mybir.dt.float32, name=f"gk{t}",
                       tag="gk")
        nc.vector.tensor_copy(out=gk[:], in_=xt[:, 0:1])
        # out = x + y; k[5] = -P so channel 5 subtracts the P result
        nc.vector.tensor_sub(
            out=xv[:, :, :, 5], in0=xv[:, :, :, 5], in1=pav[:, :, :, 0]
        )
        nc.vector.tensor_add(
            out=xv[:, :, :, 10], in0=xv[:, :, :, 10], in1=pav[:, :, :, 1]
        )
        nc.vector.tensor_add(
            out=xv[:, :, :, 54], in0=xv[:, :, :, 54], in1=pbv[:, :, :, 0]
        )
        nc.vector.tensor_add(
            out=xv[:, :, :, 67], in0=xv[:, :, :, 67], in1=pbv[:, :, :, 1]
        )
        nc.sync.dma_start(out=od[t], in_=xt[:])


def build_smats(kern: np.ndarray) -> np.ndarray:
    """Pack the 10 banded h-shift matrices (block-diag over 2 images).

    S_j[k, m] = K[k-m+2, dw+2]: out row m accumulates K[dh+2,dw+2]*x[m+dh].
    """
    P = np.asarray(kern[10], np.float32)  # cross;  kern[5] == -P
    Q = np.asarray(kern[54], np.float32)  # diag;   kern[67] == Q
    terms = [(P, dw) for dw in DWS] + [(Q, dw) for dw in DWS]
    S = np.zeros((ROWS, N_MATS * ROWS), np.float32)
    for j, (K, dw) in enumerate(terms):
        s = np.zeros((H, H), np.float32)
        for dh in (-2, -1, 0, 1, 2):
            v = K[dh + 2, dw + 2]
            if v != 0.0:
                # s[k=m+dh, m] = v
                idx = np.arange(max(0, -dh), min(H, H - dh))
                s[idx + dh, idx] = v
        blk = S[:, j * ROWS : (j + 1) * ROWS]
        blk[:H, :H] = s
        blk[H:, H:] = s
    return S


_NC_CACHE = {}


def _get_nc(n_images: int, repeats: int = 1):
    key = (n_images, repeats)
    if key not in _NC_CACHE:
        _NC_CACHE[key] = build_nc(n_images, repeats=repeats)
    return _NC_CACHE[key]


def run_sharded(x: np.ndarray, kern: np.ndarray, trace: bool = False,
                repeats: int = 1):
    """Run the SPMD kernel on 8 cores; returns (out, BassKernelResults)."""
    x = np.ascontiguousarray(x, np.float32)
    n_per = x.shape[0] // N_CORES
    nc = _get_nc(n_per, repeats)
    smats = build_smats(kern)
    in_maps = [
        {"x": x[i * n_per : (i + 1) * n_per], "s_mats": smats}
        for i in range(N_CORES)
    ]
    res = run_bass_kernel_spmd(nc, in_maps, list(range(N_CORES)), trace=trace)
    out = np.concatenate([res.results[i]["out"] for i in range(N_CORES)], axis=0)
    return out, res


def kernel(x: np.ndarray, kernel: np.ndarray) -> np.ndarray:
    out, _ = run_sharded(x, kernel)
    return out

